# revision 1
# baseline (speedup 1.0000x reference)
import numpy as np

# nn_Atoms — hardcoded problem constants
B, C, N_FRAMES, N_SAMPLES = 32, 128, 128, 32768
N_HARM, N_NOTES = 16, 128
FREQ_CHANGE = 0.05
NOISE_FILT = 64
M = 8  # cores
UP = N_SAMPLES // N_FRAMES  # 256, exact upsample factor

# linear-interp weights for 128 -> 32768 (align_corners=False), precomputed
_pos = (np.arange(N_SAMPLES, dtype=np.float64) + 0.5) / UP - 0.5
_pos = np.clip(_pos, 0.0, N_FRAMES - 1)
_W = (_pos - np.floor(_pos)).astype(np.float32)  # frac weight, 0 where clipped

_compiled = None


def _build():
    import jax
    import jax.numpy as jnp

    W = jnp.asarray(_W)
    HALF = UP // 2  # 128

    def leaky(v):
        return jnp.where(v >= 0, v, 0.2 * v)

    def unit(v):  # forward pass of straight-through clamp
        return jnp.clip(v, 0.0, 1.0)

    def linear_stack(v, p):
        for Wm, b in zip(p['ws'][:-1], p['bs'][:-1]):
            v = leaky(v @ Wm + b)
        return v @ p['ws'][-1] + p['bs'][-1]

    def conv1d(v, w, b):
        y = jax.lax.conv_general_dilated(
            v, w, (1,), 'SAME', dimension_numbers=('NCH', 'OIH', 'NCH'))
        return y + b[None, :, None]

    def conv_upsample(v, p):
        h = (v @ p['lin_w'] + p['lin_b']).reshape(v.shape[0], C, 8)
        for Wm, b in zip(p['conv_ws'], p['conv_bs']):
            h = jnp.repeat(h, 2, axis=-1)
            h = leaky(conv1d(h, Wm, b))
        return conv1d(h, p['out_w'], p['out_b'])

    def lin_interp256(v):  # (..., 128) -> (..., 32768), gather-free
        xr = jnp.repeat(v, UP, axis=-1)
        a = jnp.concatenate(
            [jnp.repeat(v[..., :1], HALF, axis=-1), xr[..., :-HALF]], axis=-1)
        b = jnp.concatenate(
            [xr[..., HALF:], jnp.repeat(v[..., -1:], HALF, axis=-1)], axis=-1)
        return a * (1.0 - W) + b * W

    def fwd(x, noise_base, gumbel_noise, params, center_freqs, harmonics):
        # hard gumbel-softmax note selection (forward = one-hot argmax)
        z = linear_stack(x, params['f0']) + gumbel_noise
        hard = jax.nn.one_hot(jnp.argmax(z, axis=-1), N_NOTES, dtype=z.dtype)
        f0 = (hard @ center_freqs)[:, None, None]

        f0_change = jnp.tanh(conv_upsample(x, params['f0_change']))
        freq = f0 + f0 * FREQ_CHANGE * f0_change
        harm = freq * harmonics[None, :, None]
        all_tones = jnp.concatenate([freq, harm], axis=1)
        all_tones = jnp.where(all_tones >= 1.0, 0.0, all_tones)
        all_tones = lin_interp256(all_tones)
        osc = jnp.sin(jnp.cumsum(all_tones * jnp.pi, axis=-1))

        # fft_convolve(noise, filt) with a 64-tap filter == causal FIR
        noise = noise_base * params['noise_level']
        filt64 = osc[:, :, :NOISE_FILT]                       # (b,17,64)
        nz = noise[:, 0, :]                                   # (b,N)
        sh = jnp.stack(
            [jnp.pad(nz[:, :N_SAMPLES - k], ((0, 0), (k, 0)))
             for k in range(NOISE_FILT)], axis=1)             # (b,64,N)
        bl_noise = jnp.einsum('bhk,bkn->bhn', filt64, sh)

        mix = lin_interp256(unit(conv_upsample(x, params['mix'])))

        amp = jax.nn.relu(conv_upsample(x, params['amp']))
        factors = unit(linear_stack(x, params['spec_shape']))[:, :, None] * amp
        amp = jnp.concatenate([amp, factors], axis=1)
        decay = 0.8 + 0.2 * unit(conv_upsample(x, params['decay']))
        prev = jnp.pad(amp[:, :, :-1], ((0, 0), (0, 0), (1, 0)))
        awd = lin_interp256(amp + prev * decay)

        full = osc * awd * mix + bl_noise * awd * (1.0 - mix)
        return jnp.mean(full, axis=1, keepdims=True)

    return jax.pmap(fwd, in_axes=(0, 0, 0, None, None, None))


def kernel(x, params, center_freqs, harmonics, noise_base, gumbel_noise):
    global _compiled
    import jax
    if _compiled is None:
        _compiled = _build()
    x = np.asarray(x).reshape(M, B // M, C)
    nb = np.asarray(noise_base).reshape(M, B // M, 1, N_SAMPLES)
    gn = np.asarray(gumbel_noise).reshape(M, B // M, N_NOTES)
    out = _compiled(x, nb, gn, params,
                    np.asarray(center_freqs), np.asarray(harmonics))
    return np.asarray(out).reshape(B, 1, N_SAMPLES).astype(np.float32)
